# revision 1
# baseline (speedup 1.0000x reference)
"""Trainium2 Bass kernel for nn_BIMM1D (Gaussian-mixture NLL loss).

Math: loss = -(1/M) sum_m log p(u_m),
  p(u) = (1/(sn*sqrt(2pi))) * S(u),
  S(u) = sum_j w_j exp(-0.5*((u - c_j)/sn)^2)
over 772 atoms (4 interior centers I_k, plus 6 interfaces x 128 MC centers
In[p,n], the latter weighted w_{4+p}/N).  All atoms are shared by every data
point, so S(.) is a fixed 1-D function: each core builds a G-node lookup
table of S on device (2 ACT passes over 7 x [128 atoms, G nodes]), then
evaluates its 32768-point shard by GPSIMD ap_gather of (value, slope) pairs
+ linear interpolation, log, and reduction.  Data-parallel over 8 cores
(u sharded, params replicated); host adds the 8 partial scalars.

Everything data-dependent is computed on device (erf for MC centers,
log_softmax of W, the table, interpolation, logs, sums).  Host supplies only
layout constants (arange / identity / one-hot selectors / ones).
"""
import os
import sys
import math
import numpy as np

for _p in ("/opt/trn_rl_repo", "/root/.axon_site/_ro/trn_rl_repo"):
    if os.path.isdir(_p) and _p not in sys.path:
        sys.path.insert(0, _p)

import concourse.bass as bass
import concourse.bacc as bacc
import concourse.mybir as mybir
import concourse.tile as tile
from concourse.bass_utils import run_bass_kernel_spmd
from contextlib import ExitStack

dt = mybir.dt
AF = mybir.ActivationFunctionType
ALU = mybir.AluOpType

# ---- static problem geometry (hardcoded per contract) ----
M_TOTAL = 262144
N_CORES = 8
M_SHARD = M_TOTAL // N_CORES          # 32768
N_MC = 128                            # MC samples per interface
N_PAIRS = 6
N_PHASES = 4
N_GROUPS = 7                          # 6 interface groups + 1 interior group
NJ = M_SHARD // 8                     # 4096 points per gpsimd-core group
LOG_2PI = math.log(2.0 * math.pi)

# lookup grid (covers u in [0,1) with margin; indices clamped to [1, G-2])
G = 256
GRID_LO = -0.0625
GRID_HI = 1.0625
H = (GRID_HI - GRID_LO) / (G - 1)
INV_H = 1.0 / H
SQRT2 = math.sqrt(2.0)

PACK4 = False
_IA = [0, 0, 0, 1, 1, 2]
_IB = [1, 2, 3, 2, 3, 3]

_cache = {}
last_exec_time_ns = None
last_results = None


def _build_nc(repeat=1, ablate=()):
    ablate = set(ablate)
    nc = bacc.Bacc("TRN2", target_bir_lowering=False, debug=False)
    f32 = dt.float32

    # --- DRAM tensors (ExternalInput / ExternalOutput) ---
    u_d = nc.dram_tensor("u", [M_SHARD], f32, kind="ExternalInput")
    uw_d = nc.dram_tensor("uw", [128, M_SHARD // 128], f32, kind="ExternalInput")
    eps_d = nc.dram_tensor("eps", [N_PAIRS, N_MC], f32, kind="ExternalInput")
    i4_d = nc.dram_tensor("I4", [N_PHASES, 1], f32, kind="ExternalInput")
    sncol_d = nc.dram_tensor("sncol", [128, 1], f32, kind="ExternalInput")
    dcol_d = nc.dram_tensor("dcolin", [128, 1], f32, kind="ExternalInput")
    w_d = nc.dram_tensor("W", [1, N_PHASES + N_PAIRS], f32, kind="ExternalInput")
    ar_d = nc.dram_tensor("arange", [G], f32, kind="ExternalInput")
    onesr_d = nc.dram_tensor("ones_row", [1, 128], f32, kind="ExternalInput")
    onesc_d = nc.dram_tensor("ones_col", [128, 1], f32, kind="ExternalInput")
    id6_d = nc.dram_tensor("ident6", [N_PAIRS, N_PAIRS], f32, kind="ExternalInput")
    sela_d = nc.dram_tensor("sela", [N_PHASES, N_PAIRS], f32, kind="ExternalInput")
    selb_d = nc.dram_tensor("selb", [N_PHASES, N_PAIRS], f32, kind="ExternalInput")
    seli_d = nc.dram_tensor("seli", [N_PHASES, 128], f32, kind="ExternalInput")
    dum_d = nc.dram_tensor("dummymask", [1, 128], f32, kind="ExternalInput")
    out_d = nc.dram_tensor("out", [1, 1], f32, kind="ExternalOutput")

    with tile.TileContext(nc) as tc, ExitStack() as ctx:
        cpool = ctx.enter_context(tc.tile_pool(name="consts", bufs=1))
        wpool = ctx.enter_context(tc.tile_pool(name="work", bufs=1))
        gpool = ctx.enter_context(tc.tile_pool(name="gwork", bufs=2))
        pp = ctx.enter_context(tc.tile_pool(name="ps", bufs=2, space="PSUM"))
        ppB = ctx.enter_context(tc.tile_pool(name="psB", bufs=2, space="PSUM"))
        ppT = ctx.enter_context(tc.tile_pool(name="psT", bufs=1, space="PSUM"))

        onesr_t = cpool.tile([1, 128], f32, tag="onesr")
        nc.sync.dma_start(onesr_t[:], onesr_d.ap())
        onesc_t = cpool.tile([128, 1], f32, tag="onesc")
        nc.sync.dma_start(onesc_t[:], onesc_d.ap())
        id6_t = cpool.tile([N_PAIRS, N_PAIRS], f32, tag="id6")
        nc.sync.dma_start(id6_t[:], id6_d.ap())
        sela_t = cpool.tile([N_PHASES, N_PAIRS], f32, tag="sela")
        nc.sync.dma_start(sela_t[:], sela_d.ap())
        selb_t = cpool.tile([N_PHASES, N_PAIRS], f32, tag="selb")
        nc.sync.dma_start(selb_t[:], selb_d.ap())
        seli_t = cpool.tile([N_PHASES, 128], f32, tag="seli")
        nc.sync.dma_start(seli_t[:], seli_d.ap())
        dum_t = cpool.tile([1, 128], f32, tag="dum")
        nc.sync.dma_start(dum_t[:], dum_d.ap())
        # node coordinates replicated to all partitions: [128, G] of 0..G-1
        # (pure layout constant -> loaded once, outside the repeat loop)
        xrep_t = cpool.tile([128, G], f32, tag="xrep")
        nc.sync.dma_start(
            xrep_t[:],
            ar_d.ap().rearrange("(a b) -> a b", a=1).to_broadcast((128, G)),
        )

        def body():
            # ---- load params + constants ----
            eps_t = cpool.tile([N_PAIRS, N_MC], f32, tag="eps")
            nc.sync.dma_start(eps_t[:], eps_d.ap())
            i4_t = cpool.tile([N_PHASES, 1], f32, tag="i4")
            nc.sync.dma_start(i4_t[:], i4_d.ap())
            sncol_t = cpool.tile([128, 1], f32, tag="sncol")
            nc.sync.dma_start(sncol_t[:], sncol_d.ap())
            dcol = wpool.tile([128, 1], f32, tag="dcol")
            nc.sync.dma_start(dcol[:], dcol_d.ap())
            w_t = cpool.tile([1, N_PHASES + N_PAIRS], f32, tag="w")
            nc.sync.dma_start(w_t[:], w_d.ap())

            # ---- scalar prep (sn/d arrive pre-replicated as [128,1]) ----
            iscol = wpool.tile([128, 1], f32, tag="iscol")
            nc.vector.reciprocal(iscol[:], sncol_t[:])

            scale_erf = wpool.tile([128, 1], f32, tag="scale_erf")
            nc.vector.tensor_scalar_mul(scale_erf[:], dcol[:], SQRT2)
            bias_erf = wpool.tile([128, 1], f32, tag="bias_erf")
            nc.vector.tensor_scalar_mul(bias_erf[:], dcol[:], -1.0 / SQRT2)
            scale1 = wpool.tile([128, 1], f32, tag="scale1")
            nc.vector.tensor_scalar_mul(scale1[:], iscol[:], H / SQRT2)
            negk = wpool.tile([128, 1], f32, tag="negk")
            nc.vector.tensor_scalar_mul(negk[:], iscol[:], -1.0 / SQRT2)

            # ---- interface centers In [6, 128] (erf on ACT) ----
            e1 = wpool.tile([N_PAIRS, N_MC], f32, tag="e1")
            nc.scalar.activation(e1[:], eps_t[:], AF.Erf,
                                 bias=bias_erf[0:N_PAIRS, :], scale=scale_erf[0:N_PAIRS, :])
            iac_p = pp.tile([N_PAIRS, 1], f32, tag="smallp")
            nc.tensor.matmul(iac_p[:], sela_t[:], i4_t[:], start=True, stop=True)
            ibc_p = pp.tile([N_PAIRS, 1], f32, tag="smallp")
            nc.tensor.matmul(ibc_p[:], selb_t[:], i4_t[:], start=True, stop=True)
            iacol = wpool.tile([N_PAIRS, 1], f32, tag="iacol")
            nc.vector.tensor_copy(iacol[:], iac_p[:])
            hdiff = wpool.tile([N_PAIRS, 1], f32, tag="hdiff")
            nc.vector.tensor_tensor(hdiff[:], ibc_p[:], iacol[:], ALU.subtract)
            nc.vector.tensor_scalar_mul(hdiff[:], hdiff[:], 0.5)
            cin = wpool.tile([N_PAIRS, N_MC], f32, tag="cin")
            nc.vector.tensor_scalar(cin[:], e1[:], 1.0, hdiff[:], ALU.add, ALU.mult)
            nc.vector.tensor_scalar(cin[:], cin[:], iacol[:], None, ALU.add)

            # ---- unnormalized log-weights (Wm = W - max); ln(sum exp) is
            # folded into the output correction so Exp and Ln cluster by
            # ACT table-set.
            m11 = wpool.tile([1, 1], f32, tag="m11")
            nc.vector.reduce_max(m11[:], w_t[:], axis=mybir.AxisListType.X)
            wm = wpool.tile([1, N_PHASES + N_PAIRS], f32, tag="wm")
            nc.vector.tensor_scalar(wm[:], w_t[:], m11[:], None, ALU.subtract)
            # force the se Exp after the Erf (one sigmoid->exp set switch)
            z0 = wpool.tile([1, 1], f32, tag="z0")
            nc.vector.tensor_scalar_mul(z0[:], e1[0:1, 0:1], 0.0)
            wm2 = wpool.tile([1, N_PHASES + N_PAIRS], f32, tag="wm2")
            nc.vector.tensor_scalar(wm2[:], wm[:], z0[:], None, ALU.add)
            ee = wpool.tile([1, N_PHASES + N_PAIRS], f32, tag="ee")
            se = wpool.tile([1, 1], f32, tag="se")
            nc.scalar.activation(ee[:], wm2[:], AF.Exp, accum_out=se[:])
            lsm = wm
            lwrow = wpool.tile([1, N_GROUPS], f32, tag="lwrow")
            nc.vector.memset(lwrow[:], 0.0)
            nc.vector.tensor_scalar(lwrow[0:1, 0:N_PAIRS], lsm[0:1, N_PHASES:],
                                    math.log(float(N_MC)), None, ALU.subtract)
            neg_t = wpool.tile([1, 1], f32, tag="neg_t")
            nc.vector.memset(neg_t[:], -1.0e30)

            # ---- assemble per-atom center / log-weight columns [128, 7] ----
            cc_p = ppT.tile([128, 8], f32, tag="cc_p")
            nc.tensor.transpose(cc_p[:, 0:N_PAIRS], cin[:], id6_t[:])
            nc.tensor.matmul(cc_p[:, N_PAIRS:N_PAIRS + 1], seli_t[:], i4_t[:],
                             start=True, stop=True)
            ccols = wpool.tile([128, N_GROUPS], f32, tag="ccols")
            nc.vector.tensor_copy(ccols[:, N_PAIRS:N_GROUPS], cc_p[:, N_PAIRS:N_GROUPS])
            nc.vector.tensor_copy(ccols[:, 0:N_PAIRS], cc_p[:, 0:N_PAIRS])

            # lsm as a column: lsmcol[10,1] = lsm.T @ [1]
            lsmc_p = pp.tile([N_PHASES + N_PAIRS, 1], f32, tag="smallp")
            nc.tensor.matmul(lsmc_p[:], lsm[:], onesr_t[0:1, 0:1], start=True, stop=True)
            lsmcol = wpool.tile([N_PHASES + N_PAIRS, 1], f32, tag="lsmcol")
            nc.vector.tensor_copy(lsmcol[:], lsmc_p[:])

            lw_p = ppT.tile([128, 8], f32, tag="lw_p")
            nc.tensor.matmul(lw_p[:, 0:N_PAIRS], onesr_t[:], lwrow[0:1, 0:N_PAIRS],
                             start=True, stop=True)
            nc.tensor.matmul(lw_p[:, N_PAIRS:N_PAIRS + 1], seli_t[:],
                             lsmcol[0:N_PHASES, :], start=True, stop=False)
            nc.tensor.matmul(lw_p[:, N_PAIRS:N_PAIRS + 1], dum_t[:], neg_t[:],
                             start=False, stop=True)
            lw = wpool.tile([128, N_GROUPS], f32, tag="lw")
            nc.vector.tensor_copy(lw[:], lw_p[:, 0:N_GROUPS])

            bias_cols = wpool.tile([128, N_GROUPS], f32, tag="bias_cols")
            nc.vector.tensor_scalar(bias_cols[:, N_PAIRS:N_GROUPS],
                                    ccols[:, N_PAIRS:N_GROUPS], GRID_LO, negk[:],
                                    ALU.subtract, ALU.mult)
            nc.vector.tensor_scalar(bias_cols[:, 0:N_PAIRS], ccols[:, 0:N_PAIRS],
                                    GRID_LO, negk[:], ALU.subtract, ALU.mult)

            # ---- build table: T[g] = sum_j w_j exp(-0.5 t^2) over 7 groups ----
            pT0 = ppT.tile([1, G // 2], f32, tag="pT0")
            pT1 = ppT.tile([1, G // 2], f32, tag="pT1")
            n_groups_eff = 1 if "table1" in ablate else N_GROUPS
            group_order = list(range(n_groups_eff))
            if n_groups_eff == N_GROUPS:
                group_order = [N_PAIRS] + list(range(N_PAIRS))
            for gi, g in enumerate(group_order):
                s1 = gpool.tile([128, G], f32, tag="s1")
                nc.scalar.activation(s1[:], xrep_t[:], AF.Square,
                                     bias=bias_cols[:, g:g + 1], scale=scale1[:])
                eg = gpool.tile([128, G], f32, tag="eg")
                nc.scalar.activation(eg[:], s1[:], AF.Exp,
                                     bias=lw[:, g:g + 1], scale=-1.0)
                nc.tensor.matmul(pT0[:], onesc_t[:], eg[:, 0:G // 2],
                                 start=(gi == 0), stop=(gi == n_groups_eff - 1))
                nc.tensor.matmul(pT1[:], onesc_t[:], eg[:, G // 2:G],
                                 start=(gi == 0), stop=(gi == n_groups_eff - 1))
            trow = wpool.tile([1, G], f32, tag="trow")
            nc.vector.tensor_copy(trow[0:1, 0:G // 2], pT0[:])
            nc.vector.tensor_copy(trow[0:1, G // 2:G], pT1[:])

            # pair row: [T[g], 0.5*(T[g+1]-T[g-1])] interleaved
            pairrow = wpool.tile([1, 2 * G], f32, tag="pairrow")
            nc.vector.memset(pairrow[0:1, 1:2], 0.0)
            nc.vector.memset(pairrow[0:1, 2 * G - 1:2 * G], 0.0)
            nc.vector.tensor_copy(pairrow[0:1, 0:2 * G:2], trow[:])
            nc.vector.tensor_tensor(pairrow[0:1, 3:2 * G - 1:2],
                                    trow[0:1, 2:G], trow[0:1, 0:G - 2], ALU.subtract)
            nc.vector.tensor_scalar_mul(pairrow[0:1, 3:2 * G - 1:2],
                                        pairrow[0:1, 3:2 * G - 1:2], 0.5)

            # replicate pair table to all 128 partitions
            tbl = wpool.tile([128, 2 * G], f32, tag="tbl")
            for i in range(2 * G // 512):
                ptb = ppB.tile([128, 512], f32, tag="ptb")
                nc.tensor.matmul(ptb[:], onesr_t[:], pairrow[0:1, 512 * i:512 * (i + 1)],
                                 start=True, stop=True)
                nc.scalar.copy(tbl[:, 512 * i:512 * (i + 1)], ptb[:])

            # ---- wrap-layout u -> int16 gather indices ----
            u_wrap = wpool.tile([128, M_SHARD // 128], f32, tag="u_wrap")
            sw = M_SHARD // 128  # 256 columns
            nc.sync.dma_start(u_wrap[:], uw_d.ap())
            tw = wpool.tile([128, sw], f32, tag="tw")
            nc.vector.tensor_scalar(tw[:], u_wrap[:], GRID_LO, INV_H,
                                    ALU.subtract, ALU.mult)
            nc.vector.tensor_scalar(tw[:], tw[:], 1.0, float(G - 2), ALU.max, ALU.min)
            idx16 = wpool.tile([128, sw], dt.int16, tag="idx16")
            if PACK4:
                nc.vector.tensor_scalar_mul(tw[:], tw[:], 0.5)
            nc.vector.tensor_copy(idx16[:], tw[:])

            # ---- gather (value, slope) pairs ----
            dst = wpool.tile([128, 2 * NJ], f32, tag="dst")
            if "no_gather" in ablate:
                nc.vector.memset(dst[:], 1.0)
                nc.vector.tensor_scalar_add(dst[0:1, 0:1], idx16[0:1, 0:1], 0.0)
                nc.vector.tensor_scalar_add(dst[0:1, 1:2], tbl[0:1, 0:1], 0.0)
            else:
                half = NJ // 2  # idx cols feed halves in j = s*16+p order
                nc.gpsimd.ap_gather(dst[:, 0:NJ], tbl[:], idx16[:, 0:half // 16],
                                    channels=128, num_elems=G, d=2, num_idxs=half)
                nc.gpsimd.ap_gather(dst[:, NJ:2 * NJ], tbl[:],
                                    idx16[:, half // 16:NJ // 16],
                                    channels=128, num_elems=G, d=2, num_idxs=half)

            # ---- replicated-layout interpolation chain ----
            u_rep = wpool.tile([128, NJ], f32, tag="u_rep")
            if "rep_contig" in ablate:
                u_view = u_d.ap().rearrange("(p s) -> p s", p=8)
                for k in range(8):
                    nc.sync.dma_start(u_rep[16 * k:16 * k + 8, :], u_view)
                    nc.sync.dma_start(u_rep[16 * k + 8:16 * k + 16, :], u_view)
            else:
                for k in range(8):
                    src_k = u_d.ap()[k * NJ:(k + 1) * NJ].rearrange(
                        "(a b) -> a b", a=1).to_broadcast((16, NJ))
                    nc.sync.dma_start(u_rep[16 * k:16 * (k + 1), :], src_k)
            tr = wpool.tile([128, NJ], f32, tag="tr")
            nc.vector.tensor_scalar(tr[:], u_rep[:], GRID_LO, INV_H,
                                    ALU.subtract, ALU.mult)
            trc = wpool.tile([128, NJ], f32, tag="trc")
            nc.vector.tensor_scalar(trc[:], tr[:], 1.0, float(G - 2), ALU.max, ALU.min)
            i16r = wpool.tile([128, NJ], dt.int16, tag="i16r")
            nc.vector.tensor_copy(i16r[:], trc[:])
            ifr = wpool.tile([128, NJ], f32, tag="ifr")
            nc.vector.tensor_copy(ifr[:], i16r[:])
            # frac -> reuse tr;  then lerp+log per gather half so the DVE/ACT
            # tail overlaps the second ap_gather
            nc.vector.tensor_tensor(tr[:], trc[:], ifr[:], ALU.subtract)
            logr = wpool.tile([128, NJ], f32, tag="logr")
            acc0 = wpool.tile([128, 1], f32, tag="acc0")
            acc1 = wpool.tile([128, 1], f32, tag="acc1")
            accs = [acc0, acc1]
            if "no_repchain" in ablate:
                for a in accs:
                    nc.vector.memset(a[:], 1.0)
            else:
                for h, acch in enumerate(accs):
                    lo, hi = h * (NJ // 2), (h + 1) * (NJ // 2)
                    nc.vector.tensor_tensor(ifr[:, lo:hi], tr[:, lo:hi],
                                            dst[:, 2 * lo + 1:2 * hi:2], ALU.mult)
                    nc.vector.tensor_tensor(trc[:, lo:hi], ifr[:, lo:hi],
                                            dst[:, 2 * lo:2 * hi:2], ALU.add)
                    nc.scalar.activation(logr[:, lo:hi], trc[:, lo:hi], AF.Ln,
                                         accum_out=acch[:])

            pout = pp.tile([1, 1], f32, tag="smallp")
            for h, acch in enumerate(accs):
                nc.tensor.matmul(pout[:], acch[:], onesc_t[:],
                                 start=(h == 0), stop=(h == 1))
            # ln(se), gated after the last table-build exp so the ACT queue
            # runs [Erf][Exp/Square...][Ln, Ln] with one load per set
            z1 = wpool.tile([1, 1], f32, tag="z1")
            nc.vector.tensor_scalar_mul(z1[:], eg[0:1, 0:1], 0.0)
            se2 = wpool.tile([1, 1], f32, tag="se2")
            nc.vector.tensor_scalar(se2[:], se[:], z1[:], None, ALU.add)
            lnse = wpool.tile([1, 1], f32, tag="lnse")
            nc.scalar.activation(lnse[:], se2[:], AF.Ln)
            corr = wpool.tile([1, 1], f32, tag="corr")
            nc.vector.tensor_scalar_mul(corr[:], lnse[:], float(16 * M_SHARD))
            out_sb = wpool.tile([1, 1], f32, tag="out_sb")
            nc.vector.tensor_tensor(out_sb[:], pout[:], corr[:], ALU.subtract)
            nc.sync.dma_start(out_d.ap(), out_sb[:])

        if repeat == 1:
            body()
        else:
            with tc.For_i(0, repeat, 1):
                body()

    nc.compile()
    return nc


def _consts():
    ia = np.zeros((N_PHASES, N_PAIRS), np.float32)
    ib = np.zeros((N_PHASES, N_PAIRS), np.float32)
    for p, (a, b) in enumerate(zip(_IA, _IB)):
        ia[a, p] = 1.0
        ib[b, p] = 1.0
    seli = np.zeros((N_PHASES, 128), np.float32)
    for i in range(N_PHASES):
        seli[i, i] = 1.0
    dummy = np.zeros((1, 128), np.float32)
    dummy[0, N_PHASES:] = 1.0
    return {
        "arange": np.arange(G, dtype=np.float32),
        "ones_row": np.ones((1, 128), np.float32),
        "ones_col": np.ones((128, 1), np.float32),
        "ident6": np.eye(N_PAIRS, dtype=np.float32),
        "sela": ia,
        "selb": ib,
        "seli": seli,
        "dummymask": dummy,
    }


def make_in_maps(u, uniform_eps, I, sigma_n, d, W):
    """Build the 8 per-core input maps (u sharded; params + layout consts
    replicated; uw = the gather-wrap permutation of the shard)."""
    u = np.asarray(u, np.float32).reshape(M_TOTAL)
    sn_v = np.float32(np.asarray(sigma_n).reshape(-1)[0])
    d_v = np.float32(np.asarray(d).reshape(-1)[0])
    shared = {
        "eps": np.asarray(uniform_eps, np.float32).reshape(N_PAIRS, N_MC),
        "I4": np.asarray(I, np.float32).reshape(N_PHASES, 1),
        "sncol": np.full((128, 1), sn_v, np.float32),
        "dcolin": np.full((128, 1), d_v, np.float32),
        "W": np.asarray(W, np.float32).reshape(1, N_PHASES + N_PAIRS),
        **_consts(),
    }
    in_maps = []
    for c in range(N_CORES):
        m = dict(shared)
        shard = u[c * M_SHARD:(c + 1) * M_SHARD]
        m["u"] = shard.copy()
        m["uw"] = np.ascontiguousarray(
            shard.reshape(8, M_SHARD // 128, 16).transpose(0, 2, 1)
        ).reshape(128, M_SHARD // 128)
        in_maps.append(m)
    return in_maps


def kernel(u, uniform_eps, I, sigma_b, sigma_n, d, W, n_MC_components=None):
    global last_exec_time_ns, last_results
    in_maps = make_in_maps(u, uniform_eps, I, sigma_n, d, W)

    if "nc" not in _cache:
        _cache["nc"] = _build_nc()
    nc = _cache["nc"]

    trace = bool(int(os.environ.get("KERNEL_TRACE", "0")))
    res = run_bass_kernel_spmd(nc, in_maps, core_ids=list(range(N_CORES)),
                               trace=trace)
    last_results = res
    last_exec_time_ns = res.exec_time_ns

    total = sum(float(res.results[c]["out"][0, 0]) for c in range(N_CORES))
    sn_v = float(np.asarray(sigma_n).reshape(-1)[0])
    loss = -(total / 16.0) / M_TOTAL + math.log(sn_v) + 0.5 * LOG_2PI
    return np.float32(loss)



# revision 2
# speedup vs baseline: 12.2968x; 12.2968x over previous
"""Trainium2 Bass kernel for nn_BIMM1D (Gaussian-mixture NLL loss).

Replaces the v1 table-gather design (GPSIMD ap_gather, ~112us/core) with
a polynomial-moment contraction, ~11x faster end to end:
  log S(u) ~= sum_j c_j t^j,  t = 2u-1  (deg 12; loss rel err ~1.5e-4,
  gate is 2e-2), so  sum_m log S(u_m) ~= c . mu,  mu_j = sum_m t_m^j.
The c_j are fit ON DEVICE from 128 Chebyshev-node evaluations of log S
(c = Apinv @ logS(nodes); Apinv is a host layout constant -- a pure
function of the fixed node grid).  Per-point work is only the DVE power
chain (fused multiply+reduce), with no gather and no per-point log.

Same polynomial-moment contraction as v3 (see kernel3 docstring), with
the serial cross-engine tail collapsed:
 - the node table accumulates directly as a PSUM COLUMN via
   matmul(lhsT=eg, rhs=ones) per group  (no [1,G] row, no transpose)
 - per-atom log-weights ride in the exp BIAS (v2-style lw tile incl.
   -1e30 masking), so nothing downstream of ee = exp(Wm) gates the
   group loop; ee only feeds ln(se)
 - fit coeffs and moments are produced as PSUM ROWS
   (c_row = logt^T @ Apinv, mu_row = ones^T @ pcs) and consumed by one
   fused DVE multiply+accumulate; no PSUM->SBUF copies, no dot matmul
 - ACT table schedule stays pinned to 2 loads (erf set / exp+ln set 6)
 - exp args are shifted by -k^2 x so every arg <= 0 (Ln range safety),
   compensated post-ln by -k^2(x^2-x) on the node axis
"""
import os
import sys
import math
import numpy as np

for _p in ("/opt/trn_rl_repo", "/root/.axon_site/_ro/trn_rl_repo"):
    if os.path.isdir(_p) and _p not in sys.path:
        sys.path.insert(0, _p)

import concourse.bass as bass
import concourse.bacc as bacc
import concourse.mybir as mybir
import concourse.tile as tile
from concourse.bass_utils import run_bass_kernel_spmd
from contextlib import ExitStack

dt = mybir.dt
AF = mybir.ActivationFunctionType
ALU = mybir.AluOpType

M_TOTAL = 262144
N_CORES = 8
M_SHARD = M_TOTAL // N_CORES
N_MC = 128
N_PAIRS = 6
N_PHASES = 4
N_GROUPS = 7
LOG_2PI = math.log(2.0 * math.pi)
SQRT2 = math.sqrt(2.0)

G = 128
DEG = 12
NCOEF = DEG + 1
SU = M_SHARD // 128
NP_COLS = 20

_IA = [0, 0, 0, 1, 1, 2]
_IB = [1, 2, 3, 2, 3, 3]

_cache = {}
last_exec_time_ns = None
last_results = None


def _cheb_nodes():
    k = np.arange(G)
    return 0.5 + 0.5 * np.cos(np.pi * (2 * k + 1) / (2 * G))


def _apinv_T():
    xn = _cheb_nodes()
    tn = 2.0 * xn - 1.0
    A = np.stack([tn ** j for j in range(NCOEF)], axis=1)
    Apinv = np.linalg.pinv(A)
    return np.ascontiguousarray(Apinv.T).astype(np.float32)


def _erf_coeffs():
    z = np.linspace(-1.46, 1.46, 4001)
    from scipy.special import erf as _erf
    y = _erf(z)
    A = np.stack([z * (z ** 2) ** j for j in range(6)], axis=1)
    c, *_ = np.linalg.lstsq(A, y, rcond=None)
    return c


def _restrict_act_tables(oneset=False):
    import concourse.hw_specs as hw_specs
    import concourse.bacc as _bacc
    real = hw_specs.get_activation_tables.__wrapped__
    KEEP = ({"natural_log_exp_and_others"} if oneset else
            {"sigmoid_and_others", "natural_log_exp_and_others"})

    def filtered(module_arch):
        full = real(module_arch)
        return {name: (funcs if name in KEEP else set())
                for name, funcs in full.items()}
    _bacc.get_activation_tables = filtered


def _build_nc(repeat=1, ablate=("erfpoly",), dbg=False):
    ablate = set(ablate)
    _restrict_act_tables(oneset=("erfpoly" in ablate))
    nc = bacc.Bacc("TRN2", target_bir_lowering=False, debug=False)
    f32 = dt.float32

    u_d = nc.dram_tensor("u", [128, SU], f32, kind="ExternalInput")
    par_d = nc.dram_tensor("params", [128, NP_COLS], f32, kind="ExternalInput")
    xn_d = nc.dram_tensor("xnodes", [G], f32, kind="ExternalInput")
    xsq_d = nc.dram_tensor("xsqcol", [G, 1], f32, kind="ExternalInput")
    ap_d = nc.dram_tensor("apinvT", [G, NCOEF], f32, kind="ExternalInput")
    onesr_d = nc.dram_tensor("ones_row", [1, 128], f32, kind="ExternalInput")
    onesc_d = nc.dram_tensor("ones_col", [128, 1], f32, kind="ExternalInput")
    sela_d = nc.dram_tensor("sela", [N_PHASES, N_PAIRS], f32, kind="ExternalInput")
    selhd_d = nc.dram_tensor("selhd", [N_PHASES, N_PAIRS], f32, kind="ExternalInput")
    seli_d = nc.dram_tensor("seli", [N_PHASES, 128], f32, kind="ExternalInput")
    dum_d = nc.dram_tensor("dummymask", [1, 128], f32, kind="ExternalInput")
    out_d = nc.dram_tensor("out", [1, 1], f32, kind="ExternalOutput")
    if dbg:
        dbg_logt = nc.dram_tensor("dbg_logt", [G, 1], f32, kind="ExternalOutput")
        dbg_crow = nc.dram_tensor("dbg_crow", [1, NCOEF], f32, kind="ExternalOutput")
        dbg_murow = nc.dram_tensor("dbg_murow", [1, NCOEF], f32, kind="ExternalOutput")

    with tile.TileContext(nc) as tc, ExitStack() as ctx:
        cpool = ctx.enter_context(tc.tile_pool(name="consts", bufs=1))
        wpool = ctx.enter_context(tc.tile_pool(name="work", bufs=1))
        gpool = ctx.enter_context(tc.tile_pool(name="gwork", bufs=2))
        mpool = ctx.enter_context(tc.tile_pool(name="mwork", bufs=2))
        pp = ctx.enter_context(tc.tile_pool(name="ps", bufs=2, space="PSUM"))
        ppT = ctx.enter_context(tc.tile_pool(name="psT", bufs=1, space="PSUM"))
        ppS = ctx.enter_context(tc.tile_pool(name="psS", bufs=1, space="PSUM"))

        onesr_t = cpool.tile([1, 128], f32, tag="onesr")
        nc.sync.dma_start(onesr_t[:], onesr_d.ap())
        onesc_t = cpool.tile([128, 1], f32, tag="onesc")
        nc.sync.dma_start(onesc_t[:], onesc_d.ap())
        sela_t = cpool.tile([N_PHASES, N_PAIRS], f32, tag="sela")
        nc.sync.dma_start(sela_t[:], sela_d.ap())
        selhd_t = cpool.tile([N_PHASES, N_PAIRS], f32, tag="selhd")
        nc.sync.dma_start(selhd_t[:], selhd_d.ap())
        seli_t = cpool.tile([N_PHASES, 128], f32, tag="seli")
        nc.sync.dma_start(seli_t[:], seli_d.ap())
        dum_t = cpool.tile([1, 128], f32, tag="dum")
        nc.sync.dma_start(dum_t[:], dum_d.ap())
        xrep_t = cpool.tile([128, G], f32, tag="xrep")
        nc.sync.dma_start(
            xrep_t[:],
            xn_d.ap().rearrange("(a b) -> a b", a=1).to_broadcast((128, G)),
        )
        xsq_t = cpool.tile([G, 1], f32, tag="xsq")
        nc.sync.dma_start(xsq_t[:], xsq_d.ap())
        apinv_t = cpool.tile([G, NCOEF], f32, tag="apinv")
        nc.sync.dma_start(apinv_t[:], ap_d.ap())

        def body():
            par_t = wpool.tile([128, NP_COLS], f32, tag="par")
            nc.sync.dma_start(par_t[:], par_d.ap())
            u_t = wpool.tile([128, SU], f32, tag="u")
            nc.sync.dma_start(u_t[:], u_d.ap())
            epsT = par_t[:, 0:N_PAIRS]
            sncol = par_t[:, 6:7]
            dcol = par_t[:, 7:8]
            w_row = par_t[0:1, 8:18]
            i4_t = par_t[0:4, 18:19]

            # ---- DVE: scalar prep ----
            iscol = wpool.tile([128, 1], f32, tag="iscol")
            nc.vector.reciprocal(iscol[:], sncol)
            scale_erf = wpool.tile([128, 1], f32, tag="scale_erf")
            nc.vector.tensor_scalar_mul(scale_erf[:], dcol, SQRT2)
            bias_erf = wpool.tile([128, 1], f32, tag="bias_erf")
            nc.vector.tensor_scalar_mul(bias_erf[:], dcol, -1.0 / SQRT2)
            kcol = wpool.tile([128, 1], f32, tag="kcol")
            nc.vector.tensor_scalar_mul(kcol[:], iscol[:], 1.0 / SQRT2)
            negk2 = wpool.tile([128, 1], f32, tag="negk2")
            nc.vector.scalar_tensor_tensor(negk2[:], iscol[:], -0.5, iscol[:],
                                           ALU.mult, ALU.mult)
            k2col = wpool.tile([128, 1], f32, tag="k2col")
            nc.vector.tensor_scalar_mul(k2col[:], negk2[:], -1.0)
            twokcol = wpool.tile([128, 1], f32, tag="twokcol")
            nc.vector.tensor_scalar_mul(twokcol[:], kcol[:], 2.0)
            vcorr = wpool.tile([128, 1], f32, tag="vcorr")
            nc.vector.tensor_scalar(vcorr[:], xsq_t[:], negk2[:], None,
                                    ALU.mult)
            m11 = wpool.tile([1, 1], f32, tag="m11")
            nc.vector.reduce_max(m11[:], w_row, axis=mybir.AxisListType.X)
            wm = wpool.tile([1, N_PHASES + N_PAIRS], f32, tag="wm")
            nc.vector.tensor_scalar(wm[:], w_row, m11[:], None, ALU.subtract)
            lwrow = wpool.tile([1, N_PAIRS], f32, tag="lwrow")
            nc.vector.tensor_scalar(lwrow[:], wm[0:1, N_PHASES:],
                                    math.log(float(N_MC)), None, ALU.subtract)
            neg_t = wpool.tile([1, 1], f32, tag="neg_t")
            nc.vector.memset(neg_t[:], -1.0e30)

            # ---- PE: broadcasts ----
            rows_p = pp.tile([1, 2 * N_PAIRS], f32, tag="smallp")
            nc.tensor.matmul(rows_p[0:1, 0:N_PAIRS], i4_t, sela_t[:],
                             start=True, stop=True)
            nc.tensor.matmul(rows_p[0:1, N_PAIRS:2 * N_PAIRS], i4_t,
                             selhd_t[:], start=True, stop=True)
            rows_sb = wpool.tile([1, 2 * N_PAIRS], f32, tag="rows_sb")
            nc.vector.tensor_copy(rows_sb[:], rows_p[:])
            # lsmcol [10,1] = wm^T
            lsmc_p = pp.tile([N_PHASES + N_PAIRS, 1], f32, tag="smallp")
            nc.tensor.matmul(lsmc_p[:], wm[:], onesr_t[0:1, 0:1],
                             start=True, stop=True)
            lsmcol = wpool.tile([N_PHASES + N_PAIRS, 1], f32, tag="lsmcol")
            nc.vector.tensor_copy(lsmcol[:], lsmc_p[:])

            brd_p = ppT.tile([128, 16], f32, tag="brd_p")
            nc.tensor.matmul(brd_p[:, 0:12], onesr_t[:], rows_sb[:],
                             start=True, stop=True)
            nc.tensor.matmul(brd_p[:, 12:13], seli_t[:], i4_t,
                             start=True, stop=True)
            arep = brd_p[:, 0:N_PAIRS]
            hdrep = brd_p[:, N_PAIRS:12]

            # lw tile [128, 7]: interface cols = lwrow bcast; interior col =
            # Wm_k on partitions 0..3, -1e30 elsewhere.
            lw_p = ppT.tile([128, 8], f32, tag="lw_p")
            nc.tensor.matmul(lw_p[:, 0:N_PAIRS], onesr_t[:], lwrow[:],
                             start=True, stop=True)
            nc.tensor.matmul(lw_p[:, N_PAIRS:N_GROUPS], seli_t[:],
                             lsmcol[0:N_PHASES, :], start=True, stop=False)
            nc.tensor.matmul(lw_p[:, N_PAIRS:N_GROUPS], dum_t[:], neg_t[:],
                             start=False, stop=True)

            # ---- erf: ACT table or DVE polynomial ----
            e1T = wpool.tile([128, N_PAIRS], f32, tag="e1T")
            if "erfpoly" in ablate:
                EC = _erf_coeffs()
                zz = wpool.tile([128, N_PAIRS], f32, tag="zz")
                nc.vector.tensor_scalar(zz[:], epsT, scale_erf[:],
                                        bias_erf[:], ALU.mult, ALU.add)
                z2 = wpool.tile([128, N_PAIRS], f32, tag="z2")
                nc.vector.tensor_tensor(z2[:], zz[:], zz[:], ALU.mult)
                ph = wpool.tile([128, N_PAIRS], f32, tag="ph")
                nc.vector.tensor_scalar(ph[:], z2[:], float(EC[5]),
                                        float(EC[4]), ALU.mult, ALU.add)
                for cj in (EC[3], EC[2], EC[1]):
                    nc.vector.tensor_tensor(ph[:], ph[:], z2[:], ALU.mult)
                    nc.vector.tensor_scalar(ph[:], ph[:], float(cj), None,
                                            ALU.add)
                nc.vector.tensor_tensor(ph[:], ph[:], z2[:], ALU.mult)
                nc.vector.scalar_tensor_tensor(e1T[:], ph[:], float(EC[0]),
                                               zz[:], ALU.add, ALU.mult)
            else:
                nc.scalar.activation(e1T[:], epsT, AF.Erf,
                                     bias=bias_erf[:], scale=scale_erf[:])

            # ---- DVE: centers + exp-arg prep ----
            ccols = wpool.tile([128, N_GROUPS], f32, tag="ccols")
            nc.vector.scalar_tensor_tensor(ccols[:, 0:N_PAIRS], e1T[:], 1.0,
                                           hdrep, ALU.add, ALU.mult)
            nc.vector.tensor_tensor(ccols[:, 0:N_PAIRS], ccols[:, 0:N_PAIRS],
                                    arep, ALU.add)
            nc.vector.tensor_copy(ccols[:, N_PAIRS:N_GROUPS], brd_p[:, 12:13])
            z0 = wpool.tile([1, 1], f32, tag="z0")
            nc.vector.tensor_scalar_mul(z0[:], e1T[0:1, 0:1], 0.0)
            wm2 = wpool.tile([1, N_PHASES + N_PAIRS], f32, tag="wm2")
            nc.vector.tensor_scalar(wm2[:], wm[:], z0[:], None, ALU.add)
            kc = wpool.tile([128, N_GROUPS], f32, tag="kc")
            nc.vector.tensor_scalar(kc[:], ccols[:], kcol[:], None, ALU.mult)
            scale_cols = wpool.tile([128, N_GROUPS], f32, tag="scale_cols")
            nc.vector.tensor_scalar(scale_cols[:], kc[:], twokcol[:], k2col[:],
                                    ALU.mult, ALU.subtract)
            bias_cols = wpool.tile([128, N_GROUPS], f32, tag="bias_cols")
            nc.vector.scalar_tensor_tensor(bias_cols[:], kc[:], -1.0, kc[:],
                                           ALU.mult, ALU.mult)
            nc.vector.tensor_tensor(bias_cols[:], bias_cols[:],
                                    lw_p[:, 0:N_GROUPS], ALU.add)

            # ---- DVE: moments ----
            pcs = wpool.tile([128, NCOEF], f32, tag="pcs")
            t_t = wpool.tile([128, SU], f32, tag="t")
            if "pool_red" in ablate:
                nc.vector.memset(pcs[0:1, 0:1], float(M_SHARD))
            else:
                nc.vector.memset(pcs[:, 0:1], float(SU))
            if "no_moments" in ablate:
                nc.vector.memset(pcs[:, 1:NCOEF], 0.0)
                nc.vector.tensor_scalar(t_t[:], u_t[:], 2.0, -1.0,
                                        ALU.mult, ALU.add)
            elif "pool_tt" in ablate:
                # even chain on DVE (fused accum), odd muls on Pool with
                # DVE reduces
                nc.vector.tensor_scalar(t_t[:], u_t[:], 2.0, -1.0,
                                        ALU.mult, ALU.add)
                nc.vector.reduce_sum(pcs[:, 1:2], t_t[:],
                                     axis=mybir.AxisListType.X)
                evens = {1: t_t}
                for j in (2, 4, 6, 8, 10, 12):
                    cur = mpool.tile([128, SU], f32, tag=f"pw{(j // 2) % 2}")
                    nc.vector.scalar_tensor_tensor(
                        cur[:], evens[j - 2] if j > 2 else t_t, 1.0,
                        t_t[:] if j == 2 else evens[2][:],
                        ALU.mult, ALU.mult, accum_out=pcs[:, j:j + 1])
                    evens[j] = cur
                for j in (3, 5, 7, 9, 11):
                    od = mpool.tile([128, SU], f32, tag=f"po{(j // 2) % 2}")
                    nc.gpsimd.tensor_tensor(od[:], evens[j - 1][:], t_t[:],
                                            ALU.mult)
                    nc.vector.reduce_sum(pcs[:, j:j + 1], od[:],
                                         axis=mybir.AxisListType.X)
            elif "pool_red" in ablate:
                # muls on DVE, full reduces on Pool into a [1, NCOEF] row
                nc.vector.tensor_scalar(t_t[:], u_t[:], 2.0, -1.0,
                                        ALU.mult, ALU.add)
                nc.gpsimd.reduce_sum(pcs[0:1, 1:2], t_t[:],
                                     axis=mybir.AxisListType.XYZWC)
                prev = t_t
                for j in range(2, NCOEF):
                    cur = mpool.tile([128, SU], f32, tag=f"pw{j % 2}")
                    nc.vector.tensor_tensor(cur[:], prev[:], t_t[:], ALU.mult)
                    nc.gpsimd.reduce_sum(pcs[0:1, j:j + 1], cur[:],
                                         axis=mybir.AxisListType.XYZWC)
                    prev = cur
            else:
                nc.vector.tensor_scalar(t_t[:], u_t[:], 2.0, -1.0,
                                        ALU.mult, ALU.add)
                nc.vector.reduce_sum(pcs[:, 1:2], t_t[:],
                                     axis=mybir.AxisListType.X)
                prev = t_t
                for j in range(2, NCOEF):
                    cur = mpool.tile([128, SU], f32, tag=f"pw{j % 2}")
                    nc.vector.scalar_tensor_tensor(
                        cur[:], prev[:], 1.0, t_t[:], ALU.mult, ALU.mult,
                        accum_out=pcs[:, j:j + 1])
                    prev = cur

            # ---- ACT + PE: node eval straight into a PSUM column ----
            pTc = ppT.tile([G, 1], f32, tag="pTc")
            n_groups_eff = 1 if "table1" in ablate else N_GROUPS
            group_order = list(range(n_groups_eff))
            if n_groups_eff == N_GROUPS:
                group_order = [N_PAIRS] + list(range(N_PAIRS))
            eg = None
            for gi, g in enumerate(group_order):
                eg = gpool.tile([128, G], f32, tag="eg")
                nc.scalar.activation(eg[:], xrep_t[:], AF.Exp,
                                     bias=bias_cols[:, g:g + 1],
                                     scale=scale_cols[:, g:g + 1])
                nc.tensor.matmul(pTc[:], eg[:], onesc_t[:],
                                 start=(gi == 0), stop=(gi == n_groups_eff - 1))

            # ---- ACT: ee (only feeds ln(se)), then lnraw ----
            ee = wpool.tile([1, N_PHASES + N_PAIRS], f32, tag="ee")
            se = wpool.tile([1, 1], f32, tag="se")
            nc.scalar.activation(ee[:], wm2[:], AF.Exp, accum_out=se[:])
            lnraw = wpool.tile([G, 1], f32, tag="lnraw")
            nc.scalar.activation(lnraw[:], pTc[:], AF.Ln)

            # ---- tail: logt (DVE), c_row/mu_row (PE), fused dot (DVE) ----
            logt = wpool.tile([G, 1], f32, tag="logt")
            nc.vector.tensor_scalar(logt[:], lnraw[:], vcorr[:], None, ALU.add)
            fitp = ppS.tile([1, 2 * NCOEF], f32, tag="fitp")
            crow = fitp[0:1, 0:NCOEF]
            murow = fitp[0:1, NCOEF:2 * NCOEF]
            nc.tensor.matmul(crow, logt[:], apinv_t[:], start=True, stop=True)
            if "pool_red" in ablate:
                mu_sbrow = pcs[0:1, :]
            else:
                nc.tensor.matmul(murow, onesc_t[:], pcs[:], start=True,
                                 stop=True)
                mu_sbrow = wpool.tile([1, NCOEF], f32, tag="mu_sbrow")
                nc.vector.tensor_copy(mu_sbrow[:], murow)
            prod = wpool.tile([1, NCOEF], f32, tag="prod")
            dot = wpool.tile([1, 1], f32, tag="dot")
            nc.vector.scalar_tensor_tensor(prod[:], mu_sbrow, 1.0, crow,
                                           ALU.mult, ALU.mult,
                                           accum_out=dot[:])

            z1 = wpool.tile([1, 1], f32, tag="z1")
            nc.vector.tensor_scalar_mul(z1[:], eg[0:1, 0:1], 0.0)
            se2 = wpool.tile([1, 1], f32, tag="se2")
            nc.vector.tensor_scalar(se2[:], se[:], z1[:], None, ALU.add)
            lnse = wpool.tile([1, 1], f32, tag="lnse")
            nc.scalar.activation(lnse[:], se2[:], AF.Ln)
            corr = wpool.tile([1, 1], f32, tag="corr")
            nc.vector.tensor_scalar_mul(corr[:], lnse[:], float(M_SHARD))
            out_sb = wpool.tile([1, 1], f32, tag="out_sb")
            nc.vector.tensor_tensor(out_sb[:], dot[:], corr[:], ALU.subtract)
            nc.sync.dma_start(out_d.ap(), out_sb[:])
            if dbg:
                nc.sync.dma_start(dbg_logt.ap(), logt[:])
                nc.sync.dma_start(dbg_crow.ap(), prod[:])
                nc.sync.dma_start(dbg_murow.ap(), pcs[0:1, :])

        if repeat == 1:
            body()
        else:
            with tc.For_i(0, repeat, 1):
                body()

    nc.compile()
    return nc


def _consts():
    ia = np.zeros((N_PHASES, N_PAIRS), np.float32)
    hd = np.zeros((N_PHASES, N_PAIRS), np.float32)
    for p, (a, b) in enumerate(zip(_IA, _IB)):
        ia[a, p] = 1.0
        hd[a, p] -= 0.5
        hd[b, p] += 0.5
    seli = np.zeros((N_PHASES, 128), np.float32)
    for i in range(N_PHASES):
        seli[i, i] = 1.0
    dummy = np.zeros((1, 128), np.float32)
    dummy[0, N_PHASES:] = 1.0
    xn = _cheb_nodes()
    return {
        "xnodes": xn.astype(np.float32),
        "xsqcol": (xn ** 2 - xn).astype(np.float32).reshape(G, 1),
        "apinvT": _apinv_T(),
        "ones_row": np.ones((1, 128), np.float32),
        "ones_col": np.ones((128, 1), np.float32),
        "sela": ia,
        "selhd": hd,
        "seli": seli,
        "dummymask": dummy,
    }


def make_in_maps(u, uniform_eps, I, sigma_n, d, W):
    u = np.asarray(u, np.float32).reshape(M_TOTAL)
    sn_v = np.float32(np.asarray(sigma_n).reshape(-1)[0])
    d_v = np.float32(np.asarray(d).reshape(-1)[0])
    par = np.zeros((128, NP_COLS), np.float32)
    par[:, 0:N_PAIRS] = np.asarray(uniform_eps, np.float32).reshape(
        N_PAIRS, N_MC).T
    par[:, 6] = sn_v
    par[:, 7] = d_v
    par[0, 8:18] = np.asarray(W, np.float32).reshape(-1)
    par[0:4, 18] = np.asarray(I, np.float32).reshape(-1)
    shared = {"params": par, **_consts()}
    in_maps = []
    for c in range(N_CORES):
        m = dict(shared)
        m["u"] = u[c * M_SHARD:(c + 1) * M_SHARD].reshape(128, SU).copy()
        in_maps.append(m)
    return in_maps


def kernel(u, uniform_eps, I, sigma_b, sigma_n, d, W, n_MC_components=None):
    global last_exec_time_ns, last_results
    in_maps = make_in_maps(u, uniform_eps, I, sigma_n, d, W)

    if "nc" not in _cache:
        _cache["nc"] = _build_nc()
    nc = _cache["nc"]

    trace = bool(int(os.environ.get("KERNEL_TRACE", "0")))
    res = run_bass_kernel_spmd(nc, in_maps, core_ids=list(range(N_CORES)),
                               trace=trace)
    last_results = res
    last_exec_time_ns = res.exec_time_ns

    total = sum(float(res.results[c]["out"][0, 0]) for c in range(N_CORES))
    sn_v = float(np.asarray(sigma_n).reshape(-1)[0])
    loss = -total / M_TOTAL + math.log(sn_v) + 0.5 * LOG_2PI
    return np.float32(loss)
